# revision 6
# baseline (speedup 1.0000x reference)
"""GQA causal attention kernel for 8 Trainium2 NeuronCores.

Sharding: core c -> (batch b = c//2, kv-head pair p = c%2). Each core computes
its batch's attention for 2 kv heads (8 q heads) plus the partial output
projection over its 1024 hd columns of Wp; host adds the two partials per batch.

Layout strategy (all transposes done on HOST in numpy):
  device receives xT [C, T] bf16, W*T pre-transposed bf16. On device:
  QT/KT computed directly in [d, t] layout, V in natural [t, d] layout.
  scoresT[tk, tq] = KT.T @ QT per (kv-tile, tq-tile); exp on ACT (no max
  subtraction -- scores are O(6) so exp is safe in f32/bf16); causal masking
  via 0/1 mask multiply post-exp; PV matmul: O^T[d, tq] += V.T @ expT.
  Softmax denominators via N=1 mini-matmuls (expT-chunk as stationary, ones
  column moving) giving per-tq columns; reciprocal on DVE; transposed to a row
  via PE transpose; broadcast across partitions via K=1 outer-product matmul;
  applied to O at flush. RoPE in [d, t] layout via a partition-half swap
  (SBUF->SBUF DMA) and sign-folded sin/cos tables (scale 1/sqrt(128) folded
  into the Q tables).
"""

import sys

sys.path.insert(0, "/opt/trn_rl_repo")

import numpy as np
import ml_dtypes

import concourse.bass as bass
import concourse.bacc as bacc
import concourse.tile as tile
import concourse.mybir as mybir
from concourse.bass_utils import run_bass_kernel_spmd

BF16 = mybir.dt.bfloat16
F32 = mybir.dt.float32
NPBF = ml_dtypes.bfloat16

P = 128          # partitions / head size
T = 2048         # sequence length
C = 2048         # embed dim
CT = C // P      # 16 contraction tiles
NQ = 8           # local q heads per core
NKV = 2          # local kv heads per core
TQ = 4           # tq tiles of 512
TQW = 512
NT = T // P      # 16 t-tiles of 128
HD = NQ * P      # 1024 local hd
N_CORES = 8

_COMPILED = None


def _build():
    nc = bacc.Bacc("TRN2", target_bir_lowering=False, debug=False)

    xT_d = nc.dram_tensor("xT", [C, T], BF16, kind="ExternalInput")
    wqT_d = nc.dram_tensor("wqT", [C, HD], BF16, kind="ExternalInput")
    wkT_d = nc.dram_tensor("wkT", [C, NKV * P], BF16, kind="ExternalInput")
    wvT_d = nc.dram_tensor("wvT", [C, NKV * P], BF16, kind="ExternalInput")
    wpT_d = nc.dram_tensor("wpT", [HD, C], BF16, kind="ExternalInput")
    cq_d = nc.dram_tensor("cq", [P, T], BF16, kind="ExternalInput")
    sq_d = nc.dram_tensor("sq", [P, T], BF16, kind="ExternalInput")
    ck_d = nc.dram_tensor("ck", [P, T], BF16, kind="ExternalInput")
    sk_d = nc.dram_tensor("sk", [P, T], BF16, kind="ExternalInput")
    masks_d = nc.dram_tensor("masks", [4, P, TQW], BF16, kind="ExternalInput")
    onescol_d = nc.dram_tensor("onescol", [P, 1], BF16, kind="ExternalInput")
    onesrow_d = nc.dram_tensor("onesrow", [1, P], BF16, kind="ExternalInput")
    ident_d = nc.dram_tensor("ident", [P, P], F32, kind="ExternalInput")
    o_d = nc.dram_tensor("o", [T, C], F32, kind="ExternalOutput")

    with tile.TileContext(nc) as tc:
        with (
            tc.tile_pool(name="qkv", bufs=1) as qkvp,
        ):
            # ---- persistent outputs of phase B ----
            QT = []
            for h in range(NQ):
                t_ = qkvp.tile([P, T], BF16, tag=f"QT{h}", name=f"QT{h}")
                QT.append(t_)
            KT = []
            for g in range(NKV):
                t_ = qkvp.tile([P, T], BF16, tag=f"KT{g}", name=f"KT{g}")
                KT.append(t_)
            V = []
            for i in range(NT):
                t_ = qkvp.tile([P, NKV * P], BF16, tag=f"V{i}", name=f"V{i}")
                V.append(t_)

            # =========== phase B: QKV projections + RoPE ===========
            with (
                tc.tile_pool(name="xw", bufs=1) as xw,
                tc.tile_pool(name="tabs", bufs=1) as tabs,
                tc.tile_pool(name="bwork", bufs=3) as bwork,
                tc.tile_pool(name="psb", bufs=1, space="PSUM") as psb,
            ):
                cqt = tabs.tile([P, T], BF16)
                nc.sync.dma_start(cqt[:], cq_d.ap())
                sqt = tabs.tile([P, T], BF16)
                nc.sync.dma_start(sqt[:], sq_d.ap())
                ckt = tabs.tile([P, T], BF16)
                nc.sync.dma_start(ckt[:], ck_d.ap())
                skt = tabs.tile([P, T], BF16)
                nc.sync.dma_start(skt[:], sk_d.ap())

                xts = []
                for ct in range(CT):
                    t_ = xw.tile([P, T], BF16, tag=f"x{ct}", name=f"x{ct}")
                    nc.sync.dma_start(t_[:], xT_d.ap()[ct * P:(ct + 1) * P, :])
                    xts.append(t_)
                wqs = []
                for ct in range(CT):
                    t_ = xw.tile([P, HD], BF16, tag=f"wq{ct}", name=f"wq{ct}")
                    nc.sync.dma_start(t_[:], wqT_d.ap()[ct * P:(ct + 1) * P, :])
                    wqs.append(t_)
                wks = []
                wvs = []
                for ct in range(CT):
                    t_ = xw.tile([P, NKV * P], BF16, tag=f"wk{ct}", name=f"wk{ct}")
                    nc.sync.dma_start(t_[:], wkT_d.ap()[ct * P:(ct + 1) * P, :])
                    wks.append(t_)
                    t2 = xw.tile([P, NKV * P], BF16, tag=f"wv{ct}", name=f"wv{ct}")
                    nc.sync.dma_start(t2[:], wvT_d.ap()[ct * P:(ct + 1) * P, :])
                    wvs.append(t2)

                def rope_tile(ps, cos_t, sin_t, dst_ap, tq):
                    """ps: psum [P, TQW] f32 pre-RoPE [d, t]; writes bf16 dst."""
                    sl = slice(tq * TQW, (tq + 1) * TQW)
                    raw = bwork.tile([P, TQW], BF16, tag="raw", name="raw")
                    nc.scalar.copy(raw[:], ps[:])
                    swp = bwork.tile([P, TQW], BF16, tag="swp", name="swp")
                    nc.sync.dma_start(swp[0:64, :], raw[64:128, :])
                    nc.sync.dma_start(swp[64:128, :], raw[0:64, :])
                    m1 = bwork.tile([P, TQW], BF16, tag="m1", name="m1")
                    nc.vector.tensor_mul(m1[:], raw[:], cos_t[:, sl])
                    m2 = bwork.tile([P, TQW], BF16, tag="m2", name="m2")
                    nc.vector.tensor_mul(m2[:], swp[:], sin_t[:, sl])
                    nc.vector.tensor_add(dst_ap, m1[:], m2[:])

                # K heads first (needed by all attention tasks)
                for g in range(NKV):
                    for tq in range(TQ):
                        ps = psb.tile([P, TQW], F32, tag="psq", bufs=3, name="psk")
                        for ct in range(CT):
                            nc.tensor.matmul(
                                ps[:],
                                wks[ct][:, g * P:(g + 1) * P],
                                xts[ct][:, tq * TQW:(tq + 1) * TQW],
                                start=(ct == 0),
                                stop=(ct == CT - 1),
                            )
                        rope_tile(ps, ckt, skt, KT[g][:, tq * TQW:(tq + 1) * TQW], tq)

                # V in natural [t, d] layout
                for i in range(NT):
                    psv = psb.tile([P, NKV * P], F32, tag="psv", bufs=2, name="psv")
                    for ct in range(CT):
                        nc.tensor.matmul(
                            psv[:],
                            xts[ct][:, i * P:(i + 1) * P],
                            wvs[ct][:],
                            start=(ct == 0),
                            stop=(ct == CT - 1),
                        )
                    nc.scalar.copy(V[i][:], psv[:])

                # Q heads
                for h in range(NQ):
                    for tq in range(TQ):
                        ps = psb.tile([P, TQW], F32, tag="psq", bufs=3, name="psq")
                        for ct in range(CT):
                            nc.tensor.matmul(
                                ps[:],
                                wqs[ct][:, h * P:(h + 1) * P],
                                xts[ct][:, tq * TQW:(tq + 1) * TQW],
                                start=(ct == 0),
                                stop=(ct == CT - 1),
                            )
                        rope_tile(ps, cqt, sqt, QT[h][:, tq * TQW:(tq + 1) * TQW], tq)

            # =========== phase C: attention ===========
            with (
                tc.tile_pool(name="opool", bufs=1) as opool,
                tc.tile_pool(name="wp", bufs=1) as wpp,
                tc.tile_pool(name="const", bufs=1) as cpool,
                tc.tile_pool(name="cwork", bufs=1) as cwork,
                tc.tile_pool(name="pss", bufs=2, space="PSUM") as pss,
                tc.tile_pool(name="pso", bufs=2, space="PSUM") as pso,
                tc.tile_pool(name="psd", bufs=1, space="PSUM") as psd,
                tc.tile_pool(name="psr", bufs=1, space="PSUM") as psr,
                tc.tile_pool(name="psm", bufs=1, space="PSUM") as psm,
                tc.tile_pool(name="psw", bufs=1, space="PSUM") as psw,
                tc.tile_pool(name="dwork", bufs=3) as dwork,
            ):
                O = []
                for h in range(NQ):
                    t_ = opool.tile([P, T], BF16, tag=f"O{h}", name=f"O{h}")
                    O.append(t_)
                wpts = []
                for h in range(NQ):
                    t_ = wpp.tile([P, C], BF16, tag=f"wpt{h}", name=f"wpt{h}")
                    nc.sync.dma_start(t_[:], wpT_d.ap()[h * P:(h + 1) * P, :])
                    wpts.append(t_)
                masks = []
                for i in range(4):
                    m = cpool.tile([P, TQW], BF16, tag=f"mask{i}",
                                   name=f"mask{i}")
                    nc.sync.dma_start(m[:], masks_d.ap()[i])
                    masks.append(m)
                onescol = cpool.tile([P, 1], BF16)
                nc.sync.dma_start(onescol[:], onescol_d.ap())
                onesrow = cpool.tile([1, P], BF16)
                nc.sync.dma_start(onesrow[:], onesrow_d.ap())
                ident = cpool.tile([P, P], F32)
                nc.sync.dma_start(ident[:], ident_d.ap())

                for tq in range(TQ):
                    ktiles = (tq + 1) * 4
                    sl = slice(tq * TQW, (tq + 1) * TQW)
                    for h in range(NQ):
                        g = h // 4
                        ps_o = pso.tile([P, TQW], F32, tag="pso", name="ps_o")
                        ps_d = psd.tile([P, 4], F32, tag="psd", name="ps_d")
                        expts = []
                        # scores + exp (+mask)
                        for k in range(ktiles):
                            ps_s = pss.tile([P, TQW], F32, tag="pss", bufs=2,
                                            name="ps_s")
                            nc.tensor.matmul(
                                ps_s[:],
                                KT[g][:, k * P:(k + 1) * P],
                                QT[h][:, sl],
                                start=True,
                                stop=True,
                            )
                            ex = cwork.tile([P, TQW], BF16, tag="expt", bufs=16,
                                            name="ex")
                            nc.scalar.activation(
                                ex[:], ps_s[:], mybir.ActivationFunctionType.Exp
                            )
                            delta = k * P - tq * TQW
                            if delta >= 0:
                                nc.vector.tensor_mul(
                                    ex[:], ex[:], masks[delta // P][:]
                                )
                            expts.append(ex)
                        # PV + denominator minis
                        for k in range(ktiles):
                            nc.tensor.matmul(
                                ps_o[:],
                                V[k][:, g * P:(g + 1) * P],
                                expts[k][:],
                                start=(k == 0),
                                stop=(k == ktiles - 1),
                            )
                        # one accumulation group at a time per PSUM region
                        for c4 in range(4):
                            for k in range(ktiles):
                                nc.tensor.matmul(
                                    ps_d[:, c4:c4 + 1],
                                    expts[k][:, c4 * P:(c4 + 1) * P],
                                    onescol[:],
                                    start=(k == 0),
                                    stop=(k == ktiles - 1),
                                )
                        # denom -> reciprocal -> row -> broadcast
                        rcol = cwork.tile([P, 4], F32, tag="rcol", bufs=2,
                                          name="rcol")
                        nc.vector.reciprocal(rcol[:], ps_d[:])
                        ps_r = psr.tile([1, TQW], F32, tag="psr", name="ps_r")
                        for c4 in range(4):
                            nc.tensor.transpose(
                                ps_r[0:1, c4 * P:(c4 + 1) * P],
                                rcol[:, c4:c4 + 1],
                                ident[:],
                            )
                        rrow = cwork.tile([1, TQW], BF16, tag="rrow", bufs=2,
                                          name="rrow")
                        nc.scalar.copy(rrow[:], ps_r[:])
                        ps_m = psm.tile([P, TQW], F32, tag="psm", name="ps_m")
                        nc.tensor.matmul(
                            ps_m[:], onesrow[:], rrow[:], start=True, stop=True
                        )
                        oraw = cwork.tile([P, TQW], BF16, tag="oraw", bufs=2,
                                          name="oraw")
                        nc.scalar.copy(oraw[:], ps_o[:])
                        nc.vector.tensor_mul(O[h][:, sl], oraw[:], ps_m[:])

                # =========== phase D: output projection ===========
                for tt in range(NT):
                    for co in range(TQ):
                        ps_w = psw.tile([P, TQW], F32, tag="psw", bufs=1,
                                        name="ps_w")
                        for h in range(NQ):
                            nc.tensor.matmul(
                                ps_w[:],
                                O[h][:, tt * P:(tt + 1) * P],
                                wpts[h][:, co * TQW:(co + 1) * TQW],
                                start=(h == 0),
                                stop=(h == NQ - 1),
                            )
                        res = dwork.tile([P, TQW], F32, tag="res", name="res")
                        nc.scalar.copy(res[:], ps_w[:])
                        nc.sync.dma_start(
                            o_d.ap()[tt * P:(tt + 1) * P,
                                     co * TQW:(co + 1) * TQW],
                            res[:],
                        )

    nc.compile()
    return nc


def _prep_core_inputs(x, Wq, Wk, Wv, Wp, core):
    b = core // 2
    p = core % 2
    xT = np.ascontiguousarray(x[b].T).astype(NPBF)
    wqT = np.ascontiguousarray(Wq[p * HD:(p + 1) * HD, :].T).astype(NPBF)
    wkT = np.ascontiguousarray(Wk[p * NKV * P:(p + 1) * NKV * P, :].T).astype(NPBF)
    wvT = np.ascontiguousarray(Wv[p * NKV * P:(p + 1) * NKV * P, :].T).astype(NPBF)
    wpT = np.ascontiguousarray(Wp[:, p * HD:(p + 1) * HD].T).astype(NPBF)

    half = P // 2
    inv = (1.0 / 10000.0) ** (np.arange(half, dtype=np.float64) / half)
    ang = np.outer(inv, np.arange(T, dtype=np.float64))  # [64, T]
    cos64 = np.cos(ang)
    sin64 = np.sin(ang)
    c128 = np.concatenate([cos64, cos64], axis=0)
    s128 = np.concatenate([-sin64, sin64], axis=0)
    scale = 1.0 / np.sqrt(P)
    cq = (c128 * scale).astype(NPBF)
    sq = (s128 * scale).astype(NPBF)
    ck = c128.astype(NPBF)
    sk = s128.astype(NPBF)

    masks = np.zeros((4, P, TQW), dtype=NPBF)
    tk = np.arange(P)[:, None]
    tqi = np.arange(TQW)[None, :]
    for i in range(4):
        masks[i] = ((tk + i * P) <= tqi).astype(NPBF)

    return {
        "xT": xT, "wqT": wqT, "wkT": wkT, "wvT": wvT, "wpT": wpT,
        "cq": cq, "sq": sq, "ck": ck, "sk": sk, "masks": masks,
        "onescol": np.ones((P, 1), dtype=NPBF),
        "onesrow": np.ones((1, P), dtype=NPBF),
        "ident": np.eye(P, dtype=np.float32),
    }


def _get_compiled():
    global _COMPILED
    if _COMPILED is None:
        _COMPILED = _build()
    return _COMPILED


def _run(inputs, trace=False, tmpdir=None):
    nc = _get_compiled()
    in_maps = [
        _prep_core_inputs(inputs["x"], inputs["Wq"], inputs["Wk"],
                          inputs["Wv"], inputs["Wp"], c)
        for c in range(N_CORES)
    ]
    res = run_bass_kernel_spmd(
        nc, in_maps, list(range(N_CORES)), trace=trace,
        **({"tmpdir": tmpdir} if tmpdir else {}),
    )
    B = inputs["x"].shape[0]
    out = np.empty((B, T, C), dtype=np.float32)
    for b in range(B):
        out[b] = res.results[2 * b]["o"] + res.results[2 * b + 1]["o"]
    return out, res


def kernel(**inputs):
    out, _ = _run(inputs, trace=False)
    return out
